# revision 4
# baseline (speedup 1.0000x reference)
"""Trainium2 Bass kernel for nn_EntropyLoss (retrieval_knn).

Computes var([E(f1)-E(f0), E(f2)-E(f1)], ddof=1) where
E(f) = log(1 + sum_b sum_i r_ball[b, i]) and r_ball[b, i] is the K-th
nearest-neighbor distance (K = C//10 = 51, i.e. 52nd smallest including
the self-distance 0) among the C=512 channel vectors (dim H*W = 4096)
of sample b.

Strategy (8 NeuronCores, data-parallel over the 48 (tensor, sample)
units, 6 units per core):

  ranking key (symmetric!):  kappa_ij = G_ij - sq_i/2 - sq_j/2 + 4096
    - column bias fp16(2048 - sq_j/2) folded into the Gram as a K=1
      matmul row (PE)
    - row bias fp32(2048 - sq_i/2) folded into the PSUM->SBUF copy via
      the ScalarE activation per-partition bias
    - d2_ij = 8192 - 2 kappa_ij, so the host tail needs only kappa52.

  PE (symmetric Gram): per unit only the upper-triangle block columns:
    row-block I accumulates columns [128I:512] (widths 512/384/256/128)
    = 0.625x the matmul volume of the full Gram (the full Gram at the
    measured 260 ns/MM would be ~206 us/core, above everything else).
    Mirror tiles (J,I) = transpose of (I,J) (kappa symmetric) are
    produced by DMA transposes (dma_start_transpose, fp16) into the
    assembled per-block SBUF tile.

  Selection: per row, the 52nd largest kappa is found by an 9-step
    clipped-Newton threshold search on the count
    c_i(t) = #{j: kappa_ij >= t}:
        t' = t + clip(gain_k (c_i(t) - 51.5), +-clip_k)
    t_0 = rowbias_i + 114 (per-row recentering; the search then runs in
    the row-bias-free domain where kappa52 - rb in [88, 147]).
    Counts come from two engines in parallel:
      - DVE blocks:  tensor_scalar(is_ge) + accum_out  (~850 ns/iter)
      - ACT blocks:  activation(Sign, bias=-t) + accum_out (~1070 ns),
        sign-sum s = 2c - 512 handled in the update map.
    Updates are 3 small batched DVE ops per unit-group per iteration.
    Schedule validated on the host against exact sort: worst |t-k52|
    ~3, final var rel err ~6e-4 (gate 2e-2).

  host tail: r = sqrt(8192 - 2 t_final), then log/var in fp64.
"""
import sys

for _p in ("/opt/trn_rl_repo", "/root/.axon_site/_ro/trn_rl_repo"):
    if _p not in sys.path:
        sys.path.insert(0, _p)

import numpy as np

from concourse import bacc, mybir
from concourse.tile import TileContext
from concourse.bass_utils import run_bass_kernel_spmd

F32 = mybir.dt.float32
F16 = mybir.dt.float16
ALU = mybir.AluOpType
ACTF = mybir.ActivationFunctionType

B, C, H, W = 16, 512, 64, 64
D = H * W  # 4096
K = C // 10  # 51 -> 52nd smallest distance per row
RANK = K + 1  # 52
N_CORES = 8
N_TENSORS = 3
UNITS = N_TENSORS * B  # 48
UPC = UNITS // N_CORES  # units per core = 6
KCHUNKS = D // 128  # 32
RBLK = C // 128  # 4 row blocks per unit
NBLK = UPC * RBLK  # 24 blocks per core

# Newton schedule: (gain, clip) per iteration; search starts at
# t0 = rowbias + T0 per row. Validated on host (study_final).
T0 = 114.0
SCHEDULE = [
    (1.3, 40.0), (1.2, 20.0), (1.1, 10.0), (1.1, 6.0), (1.1, 4.0),
    (1.1, 2.0), (1.1, 1.0), (1.1, 0.5), (1.1, 0.3),
]
CNT_MID = float(RANK) - 0.5  # 51.5

# per-unit engine assignment for the selection counts:
# 'd' = DVE tensor_scalar count, 'a' = ACT sign count.
ASSIGN = ["d", "a", "d", "a", "d", "a"]

M_DT = F16  # assembled kappa tile dtype (DMA transpose keeps this fp16)
DMA_SPLIT = 4

TRACE = False
_LAST = {}


def _build_program(loop_n=None, assign=None):
    assign = assign or ASSIGN
    nc = bacc.Bacc("TRN2", target_bir_lowering=False, debug=False)

    xt_d = nc.dram_tensor(
        "xt", [UPC, 128, KCHUNKS * C], F16, kind="ExternalInput"
    )
    # col bias row: fp16(2048 - sq/2) per unit
    sqn_d = nc.dram_tensor("sqn", [UPC, C], F16, kind="ExternalInput")
    # row bias fp32 (2048 - sq/2) laid out [128, NBLK] (partition = row
    # within block, col = global block id) and the same + T0 for t init
    rb_d = nc.dram_tensor("rb", [128, 2 * NBLK], F32, kind="ExternalInput")
    tsel_d = nc.dram_tensor("tsel", [128, NBLK], F32, kind="ExternalOutput")

    kper = KCHUNKS // DMA_SPLIT
    xt_view = xt_d.ap().rearrange("s p (d k c) -> s p d k c", d=DMA_SPLIT, k=kper)

    NSTEP = len(SCHEDULE)

    with TileContext(nc) as tc:
        with (
            tc.tile_pool(name="xpool", bufs=2 * DMA_SPLIT) as xpool,
            tc.tile_pool(name="consts", bufs=1) as consts,
            tc.tile_pool(name="mpool", bufs=10) as mpool,
            tc.tile_pool(name="gps", bufs=8, space="PSUM") as gps,
        ):
            ones = consts.tile([1, 128], F16)
            nc.vector.memset(ones, 1.0)
            sqn_all = consts.tile([1, UPC * C], F16)
            nc.sync.dma_start(
                out=sqn_all, in_=sqn_d.ap().rearrange("s c -> (s c)").unsqueeze(0)
            )
            rb_all = consts.tile([128, 2 * NBLK], F32)
            nc.sync.dma_start(out=rb_all, in_=rb_d.ap())

            # threshold ping-pong tiles + scratch
            tA = consts.tile([128, NBLK], F32)
            tB = consts.tile([128, NBLK], F32)
            nc.vector.memset(tA, 0.0)
            nc.vector.memset(tB, 0.0)
            acc = consts.tile([128, NBLK], F32)
            w1 = consts.tile([128, NBLK], F32)
            w2 = consts.tile([128, NBLK], F32)
            maskd = consts.tile([128, C], M_DT)
            sgn = consts.tile([128, C], F16)
            tfin = tA if NSTEP % 2 == 0 else tB

            def pipeline_body(_iv=None):
                for s in range(UPC):
                    xparts = []
                    for d in range(DMA_SPLIT):
                        xp = xpool.tile([128, kper, C], F16, tag="xts")
                        nc.sync.dma_start(out=xp, in_=xt_view[s, :, d])
                        xparts.append(xp)

                    sqn = sqn_all[:, s * C : (s + 1) * C]
                    kind = assign[s]
                    ms = []
                    # ---- symmetric Gram + assembly ----
                    for I in range(RBLK):
                        blk = s * RBLK + I
                        W_I = C - 128 * I  # columns [128I : 512]
                        g_full = gps.tile([128, C], F32, tag="g")
                        g_ps = g_full[:, :W_I]
                        nc.tensor.matmul(
                            out=g_ps, lhsT=ones, rhs=sqn[:, 128 * I :],
                            start=True, stop=False,
                        )
                        for k in range(KCHUNKS):
                            xp = xparts[k // kper]
                            kk = k % kper
                            nc.tensor.matmul(
                                out=g_ps,
                                lhsT=xp[:, kk, 128 * I : 128 * (I + 1)],
                                rhs=xp[:, kk, 128 * I :],
                                start=False,
                                stop=(k == KCHUNKS - 1),
                            )
                        m = mpool.tile([128, C], M_DT, tag="m")
                        ms.append(m)
                        # direct columns [128I:512] with row bias
                        nc.scalar.activation(
                            out=m[:, 128 * I :], in_=g_ps,
                            func=ACTF.Identity,
                            bias=rb_all[:, blk : blk + 1],
                        )
                        # mirror columns [0:128I] from earlier blocks
                        for J in range(I):
                            nc.sync.dma_start_transpose(
                                out=m[:, 128 * J : 128 * (J + 1)],
                                in_=ms[J][:, 128 * I : 128 * (I + 1)],
                            )

                    # ---- selection: NSTEP clipped-Newton iterations ----
                    c0, c1 = s * RBLK, (s + 1) * RBLK
                    rbt = rb_all[:, NBLK + c0 : NBLK + c1]  # rb + T0
                    for it, (gain, clip) in enumerate(SCHEDULE):
                        tprev = (tB if it % 2 else tA)[:, c0:c1]
                        tnext = (tA if it % 2 else tB)[:, c0:c1]
                        if it == 0:
                            tprev = rbt
                        for I in range(RBLK):
                            blk = s * RBLK + I
                            tp = tprev[:, I : I + 1]
                            if kind == "d":
                                nc.vector.tensor_scalar(
                                    out=maskd, in0=ms[I], scalar1=tp,
                                    scalar2=1.0, op0=ALU.is_ge, op1=ALU.mult,
                                    accum_out=acc[:, blk : blk + 1],
                                )
                            else:
                                nc.scalar.activation(
                                    out=sgn, in_=ms[I], func=ACTF.Sign,
                                    bias=tp, scale=-1.0,
                                    accum_out=acc[:, blk : blk + 1],
                                )
                        accu = acc[:, c0:c1]
                        w1u = w1[:, c0:c1]
                        w2u = w2[:, c0:c1]
                        if kind == "d":
                            # w1 = (acc - 51.5) * gain, clipped, + tprev
                            nc.vector.tensor_scalar(
                                out=w1u, in0=accu, scalar1=-CNT_MID,
                                scalar2=gain, op0=ALU.add, op1=ALU.mult,
                            )
                        else:
                            # sign sum s = -(2c - 512 - z) under scale=-1:
                            # sign(t - kappa) sums to -(c_ge - c_lt)
                            # => c - 51.5 = -(s + 2*51.5 - 512)/2
                            nc.vector.tensor_scalar(
                                out=w1u, in0=accu, scalar1=2.0 * CNT_MID - 512.0,
                                scalar2=-0.5 * gain, op0=ALU.add, op1=ALU.mult,
                            )
                        nc.vector.tensor_scalar(
                            out=w2u, in0=w1u, scalar1=clip,
                            scalar2=-clip, op0=ALU.min, op1=ALU.max,
                        )
                        nc.vector.tensor_tensor(
                            out=tnext, in0=w2u, in1=tprev, op=ALU.add,
                        )

            if loop_n is not None:
                with tc.For_i(0, loop_n, 1) as _iv:
                    pipeline_body(_iv)
            else:
                pipeline_body()

            nc.sync.dma_start(out=tsel_d.ap(), in_=tfin)

    nc.compile()
    return nc


_PROGRAM = None


def _host_inputs(feats):
    """feats: [UNITS, C, D] float32 -> per-core input maps."""
    sq64 = np.einsum("ucd,ucd->uc", feats, feats, dtype=np.float64, casting="safe")
    sqn16 = (2048.0 - sq64 / 2.0).astype(np.float16)
    rb32 = (2048.0 - sq64 / 2.0).astype(np.float32)  # [U, C]

    xt = np.ascontiguousarray(
        feats.astype(np.float16)
        .transpose(0, 2, 1)
        .reshape(UNITS, KCHUNKS, 128, C)
        .transpose(0, 2, 1, 3)
        .reshape(UNITS, 128, KCHUNKS * C)
    )

    in_maps = []
    for c in range(N_CORES):
        u0 = c * UPC
        # rb laid out [128, NBLK]: col blk=(s*RBLK+I), row p = row 128I+p
        rb = np.ascontiguousarray(
            rb32[u0 : u0 + UPC].reshape(UPC, RBLK, 128).transpose(2, 0, 1)
            .reshape(128, NBLK)
        )
        rbt = np.concatenate([rb, rb + np.float32(T0)], axis=1)
        in_maps.append(
            {
                "xt": xt[u0 : u0 + UPC],
                "sqn": sqn16[u0 : u0 + UPC],
                "rb": rbt,
            }
        )
    return in_maps, sq64


def kernel(feat0, feat1, feat2):
    global _PROGRAM
    feats = np.stack(
        [np.asarray(f).reshape(B, C, D) for f in (feat0, feat1, feat2)]
    ).reshape(UNITS, C, D)

    in_maps, sq64 = _host_inputs(feats)

    if _PROGRAM is None:
        _PROGRAM = _build_program()
    out = run_bass_kernel_spmd(
        _PROGRAM, in_maps, core_ids=list(range(N_CORES)), trace=TRACE
    )
    _LAST.clear()
    _LAST["results"] = out

    # tsel[p, s*RBLK+I] = kappa52 estimate for row (128I+p) of unit s
    k52 = np.empty((UNITS, C), dtype=np.float64)
    for c in range(N_CORES):
        sel = out.results[c]["tsel"].reshape(128, UPC, RBLK)
        k52[c * UPC : (c + 1) * UPC] = sel.transpose(1, 2, 0).reshape(UPC, C)

    d2 = 8192.0 - 2.0 * k52
    r = np.sqrt(np.clip(d2, 0.0, None))
    _LAST["r"] = r
    sums = r.reshape(N_TENSORS, B * C).sum(axis=1)
    e = np.log(sums + 1.0)
    deltas = np.array([e[1] - e[0], e[2] - e[1]])
    var = deltas.var(ddof=1)
    return np.asarray(var, dtype=np.float32)
